# revision 1
# baseline (speedup 1.0000x reference)
"""Trainium2 Bass kernel for the BIMM2D mixture NLL loss (nn_BIMM2D_test_11441792876621).

Strategy (data-parallel over 8 NeuronCores, M axis sharded):
  The loss is rewritten as nll = S0 - mean_m ln p[m] with
    p[m] = sum_{c in pos|int} exp(arg[m,c]) - sum_{c in neg} exp(arg[m,c])
    arg[m,c] = sum_r feat[r,m] * coef[r,c]
  feats = {u, v, ln v, q' = 0.5 u^2/sigma_n^2 + v^2/sn2, 1}   (computed on device)
  coef  = constant [5, 1540] matrix derived from the MC samples / params (host,
          O(n_int*N) work only). Columns: 768 interface-pos, 4 interior, 768
          interface-neg terms; interface columns absorb log w_j - log N; S0=20
          keeps every exp within fp32 range (validated: args in [-173, 22]).

  Per 128-m tile on each core: one K=27 bf16 matmul group (3x512 + 1x4 free)
  computes all 1540 args into PSUM (3-way bf16 splits of data and coef rows
  reproduce fp32-accurate products), then two ScalarE Exp ops with accum_out
  produce the (interior+pos) and neg sums directly. Finale: p = acc_a - acc_b,
  Ln, reduce.
  Per-core partial sums of ln p are combined on host: one scalar per core.
"""
import math
import sys

import numpy as np

sys.path.insert(0, "/opt/trn_rl_repo")

import ml_dtypes  # noqa: E402

LOG_GAMMA_3_2 = math.log(math.gamma(1.5))
S0 = 20.0
N_COL = 1540
SPLIT_PAIRS = [(0, 0), (0, 1), (0, 2), (1, 0), (1, 1), (2, 0)]  # (data_i, coef_j)
NCORES = 8

_ERF = np.vectorize(math.erf, otypes=[np.float64])


def _bf16_split3(x):
    x = np.asarray(x, np.float32)
    d1 = x.astype(ml_dtypes.bfloat16).astype(np.float32)
    r1 = (x - d1).astype(np.float32)
    d2 = r1.astype(ml_dtypes.bfloat16).astype(np.float32)
    r2 = (r1 - d2).astype(np.float32)
    d3 = r2.astype(ml_dtypes.bfloat16).astype(np.float32)
    return [d1, d2, d3]


def _host_constants(uniform_eps, I, W, sigma_b, sigma_n, d, r):
    n_phases = I.shape[0]
    n_int, N = uniform_eps.shape
    rho = np.tanh(np.float64(r))
    sn2 = np.float64(sigma_n) ** 2 * (1.0 - rho)
    sig_eff = np.float64(sigma_n) * np.sqrt(1.0 - rho)
    sn_sq = np.float64(sigma_n) ** 2
    logW = np.asarray(W, np.float64)
    log_w = logW - (np.log(np.sum(np.exp(logW - logW.max()))) + logW.max())
    ia, ib = np.triu_indices(n_phases, k=1)
    CONST = (-np.log(np.float64(sigma_n)) - 0.5 * np.log(2 * np.pi)
             - 0.5 * np.log(sn2) - 0.5 * np.log(np.pi))

    pos = np.zeros((5, n_int * N))
    for j in range(n_int):
        Ia, Ib = np.float64(I[ia[j]]), np.float64(I[ib[j]])
        eps = np.asarray(uniform_eps[j], np.float64)
        ux = eps * 2.0 * np.float64(d) * np.float64(sigma_b) - np.float64(d) * np.float64(sigma_b)
        x = ux / (np.sqrt(2.0) * np.float64(sigma_b))
        In = (_ERF(x) + 1.0) * 0.5 * (Ib - Ia) + Ia
        G = (Ib - Ia) / np.sqrt(2.0 * np.pi * np.float64(sigma_b) ** 2) * np.exp(-(x ** 2))
        s = slice(j * N, (j + 1) * N)
        pos[0, s] = In / sn_sq
        pos[1, s] = 2.0 * G / sn2
        pos[2, s] = 1.0
        pos[3, s] = -1.0
        pos[4, s] = (CONST - np.log(G) - 0.5 * In ** 2 / sn_sq - G ** 2 / sn2
                     + log_w[n_phases + j] - np.log(N) + S0)
    neg = pos.copy()
    neg[1] = -neg[1]

    intr = np.zeros((5, n_phases))
    beta_int_const = (np.log(2.0) - LOG_GAMMA_3_2 - 3.0 * np.log(sig_eff)
                      - np.log(np.float64(sigma_n)) - 0.5 * np.log(2 * np.pi))
    for i in range(n_phases):
        intr[0, i] = np.float64(I[i]) / sn_sq
        intr[2, i] = 2.0
        intr[3, i] = -1.0
        intr[4, i] = beta_int_const + log_w[i] - 0.5 * np.float64(I[i]) ** 2 / sn_sq + S0

    coef = np.concatenate([intr, pos, neg], axis=1)
    assert coef.shape == (5, N_COL)
    return coef, sn_sq, sn2


def _build_coef_input(coef):
    rows = []
    for f in range(4):
        sp = _bf16_split3(coef[f])
        for (_, cj) in SPLIT_PAIRS:
            rows.append(sp[cj])
    sp = _bf16_split3(coef[4])
    rows += [sp[0], sp[1], sp[2]]
    block = np.stack(rows)
    out = np.zeros((128, N_COL), np.float32)
    for g in range(4):
        out[32 * g:32 * g + 27] = block
    return out.astype(ml_dtypes.bfloat16)


def _build_core_kernel_v1(nc, M_core, sn_sq, sn2, repeat=1):
    import concourse.bass as bass
    import concourse.tile as tile
    from concourse import mybir

    F32 = mybir.dt.float32
    BF16 = mybir.dt.bfloat16
    EXP = mybir.ActivationFunctionType.Exp
    LN = mybir.ActivationFunctionType.Ln
    SQUARE = mybir.ActivationFunctionType.Square
    ADD = mybir.AluOpType.add
    SUB = mybir.AluOpType.subtract

    W = M_core // 128
    FG = M_core // 4
    TPG = FG // 128
    T = M_core // 128

    if repeat < 0:  # null kernel: I/O only (for overhead calibration)
        import concourse.tile as tile2
        d_u0 = nc.dram_tensor("u", [M_core], F32, kind="ExternalInput")
        d_v0 = nc.dram_tensor("v", [M_core], F32, kind="ExternalInput")
        d_c0 = nc.dram_tensor("coef", [128, N_COL], BF16, kind="ExternalInput")
        d_o0 = nc.dram_tensor("out", [1, 1], F32, kind="ExternalOutput")
        with tile2.TileContext(nc) as tc0:
            with tc0.tile_pool(name="nul", bufs=1) as nul:
                t0 = nul.tile([1, 1], F32)
                nc.vector.memset(t0, 0.0)
                nc.gpsimd.dma_start(out=d_o0[:, :], in_=t0)
        return nc

    d_u = nc.dram_tensor("u", [M_core], F32, kind="ExternalInput")
    d_v = nc.dram_tensor("v", [M_core], F32, kind="ExternalInput")
    d_coef = nc.dram_tensor("coef", [128, N_COL], BF16, kind="ExternalInput")
    d_out = nc.dram_tensor("out", [1, 1], F32, kind="ExternalOutput")

    inv_sqrt2_sn = float(1.0 / math.sqrt(2.0 * sn_sq))
    inv_sqrt_sn2 = float(1.0 / math.sqrt(sn2))

    with tile.TileContext(nc) as tc:
        with tc.tile_pool(name="const", bufs=1) as constp, \
             tc.tile_pool(name="prep", bufs=1) as prep, \
             tc.tile_pool(name="packp", bufs=1) as packp, \
             tc.tile_pool(name="eout", bufs=3) as eoutp, \
             tc.tile_pool(name="fin", bufs=1) as finp, \
             tc.tile_pool(name="ps", bufs=2, space="PSUM") as psp:

            coef_sb = constp.tile([128, N_COL], BF16)
            nc.gpsimd.dma_start(out=coef_sb, in_=d_coef[:, :])

            pack = packp.tile([128, FG], BF16)
            nc.vector.memset(pack, 0.0)

            u2d = prep.tile([128, W], F32)
            v2d = prep.tile([128, W], F32)
            nc.gpsimd.dma_start(out=u2d, in_=d_u[:].rearrange("(p w) -> p w", w=W))
            nc.gpsimd.dma_start(out=v2d, in_=d_v[:].rearrange("(p w) -> p w", w=W))
            lv = prep.tile([128, W], F32)
            nc.scalar.activation(out=lv, in_=v2d, func=LN)
            s1 = prep.tile([128, W], F32)
            nc.scalar.activation(out=s1, in_=u2d, func=SQUARE, scale=inv_sqrt2_sn)
            s2 = prep.tile([128, W], F32)
            nc.scalar.activation(out=s2, in_=v2d, func=SQUARE, scale=inv_sqrt_sn2)
            qp = prep.tile([128, W], F32)
            nc.vector.tensor_tensor(out=qp, in0=s1, in1=s2, op=ADD)

            ones_st = prep.tile([128, W], BF16)
            nc.vector.memset(ones_st, 1.0)

            dma_engines = [nc.gpsimd, nc.sync, nc.scalar]
            n_dma = 0

            def scatter(row, src_ap):
                nonlocal n_dma
                dma_engines[n_dma % 3].dma_start(out=pack[row:128:32, :], in_=src_ap)
                n_dma += 1

            for fi, feat in enumerate([u2d, v2d, lv, qp]):
                d1 = prep.tile([128, W], BF16, tag=f"d1_{fi}")
                nc.vector.tensor_copy(out=d1, in_=feat)
                r1 = prep.tile([128, W], F32, tag=f"r1_{fi}")
                nc.vector.tensor_tensor(out=r1, in0=feat, in1=d1, op=SUB)
                d2 = prep.tile([128, W], BF16, tag=f"d2_{fi}")
                nc.vector.tensor_copy(out=d2, in_=r1)
                r2 = prep.tile([128, W], F32, tag=f"r2_{fi}")
                nc.vector.tensor_tensor(out=r2, in0=r1, in1=d2, op=SUB)
                d3 = prep.tile([128, W], BF16, tag=f"d3_{fi}")
                nc.vector.tensor_copy(out=d3, in_=r2)
                splits = [d1, d2, d3]
                for slot, (di, _) in enumerate(SPLIT_PAIRS):
                    scatter(fi * 6 + slot, splits[di][:, :])
            for rr in range(3):
                scatter(24 + rr, ones_st[:, :])

            acc_a = finp.tile([128, T], F32)
            acc_b = finp.tile([128, T], F32)
            if repeat == 0:
                nc.vector.memset(acc_a, 1.0)
                nc.vector.memset(acc_b, 0.5)

            for rep in range(repeat):
                for t in range(T):
                    g, i = divmod(t, TPG)
                    lhsT = pack[32 * g:32 * g + 27, 128 * i:128 * (i + 1)]
                    rhs_base = 32 * g
                    psum = psp.tile([128, 2048], F32, tag="args")
                    tp = (32 * g, 0)
                    for blk in range(3):
                        nc.tensor.matmul(
                            out=psum[:, 512 * blk:512 * (blk + 1)],
                            lhsT=lhsT,
                            rhs=coef_sb[rhs_base:rhs_base + 27, 512 * blk:512 * (blk + 1)],
                            start=True, stop=True, tile_position=tp)
                    nc.tensor.matmul(
                        out=psum[:, 1536:1540],
                        lhsT=lhsT,
                        rhs=coef_sb[rhs_base:rhs_base + 27, 1536:1540],
                        start=True, stop=True, tile_position=tp)
                    e_all = eoutp.tile([128, 1540], F32, tag="e_all")
                    nc.scalar.activation(out=e_all, in_=psum[:, 0:1540], func=EXP,
                                         accum_out=acc_a[:, t:t + 1])
                    nc.vector.tensor_reduce(out=acc_b[:, t:t + 1],
                                            in_=e_all[:, 772:1540], op=ADD,
                                            axis=mybir.AxisListType.X)

            # p = acc_a - 2*acc_b  (acc_a = pos+int+neg sum, acc_b = neg sum)
            two_b = finp.tile([128, T], F32)
            nc.vector.tensor_tensor(out=two_b, in0=acc_b, in1=acc_b, op=ADD)
            p_all = finp.tile([128, T], F32)
            nc.vector.tensor_tensor(out=p_all, in0=acc_a, in1=two_b, op=SUB)
            lnp = finp.tile([128, T], F32)
            nc.scalar.activation(out=lnp, in_=p_all, func=LN)
            rsum = finp.tile([128, 1], F32)
            nc.vector.tensor_reduce(out=rsum, in_=lnp, op=ADD,
                                    axis=mybir.AxisListType.X)
            row = finp.tile([1, 128], F32)
            nc.gpsimd.dma_start(out=row, in_=rsum[:, :])
            total = finp.tile([1, 1], F32)
            nc.vector.tensor_reduce(out=total, in_=row, op=ADD,
                                    axis=mybir.AxisListType.X)
            nc.gpsimd.dma_start(out=d_out[:, :], in_=total)
    return nc




_build_core_kernel = _build_core_kernel_v1


def kernel(u, v, uniform_eps, I, W, sigma_b, sigma_n, d, r):
    import jax
    import concourse.bacc as bacc
    from concourse.bass_utils import run_bass_kernel_spmd

    platforms = {dev.platform for dev in jax.devices()}
    if platforms == {"cpu"}:
        raise RuntimeError("No neuron/axon devices visible to JAX")

    u = np.asarray(u, np.float32)
    v = np.asarray(v, np.float32)
    M = u.shape[0]
    MC = M // NCORES

    coef, sn_sq, sn2 = _host_constants(
        np.asarray(uniform_eps), np.asarray(I), np.asarray(W),
        np.asarray(sigma_b), np.asarray(sigma_n), np.asarray(d), np.asarray(r))
    coef_in = _build_coef_input(coef)

    nc = bacc.Bacc()
    _build_core_kernel(nc, MC, sn_sq, sn2)
    nc.finalize()

    in_maps = [{"u": u[c * MC:(c + 1) * MC], "v": v[c * MC:(c + 1) * MC],
                "coef": coef_in} for c in range(NCORES)]
    res = run_bass_kernel_spmd(nc, in_maps, list(range(NCORES)))
    total = sum(float(res.results[c]["out"][0, 0]) for c in range(NCORES))
    nll = S0 - total / M
    return np.float32(nll)



# revision 2
# speedup vs baseline: 7.0359x; 7.0359x over previous
"""Trainium2 Bass kernel v2 for the BIMM2D mixture NLL (transposed layout).

Math: nll = S0 - mean_m ln p[m],  p[m] = sum_c s_c exp(arg[m,c]),
arg[m,c] = alpha_c*u + beta_c*v + l_c*ln v - q'(u,v) + gamma_c.
The 1536 interface MC columns are compressed to 124 quadrature columns
(4 interior + 62 pos + 62 neg): per interface, per x-segment, the empirical
MC sample measure is replaced by a K-node Gauss quadrature (Stieltjes +
Golub-Welsch); pos/neg blocks share nodes so the small-v pos/neg cancellation
keeps its accuracy. Validated on host (fp64 + device-precision sim) at build
time via a probe comparison against the exact column set, with automatic
segment-doubling / raw-set fallback (n_pass > 1) if the probe fails.

Device layout (transposed vs the row-major v1): coef [12, NCOL=128] is the
PE stationary; data streams as the moving operand, 3-group batches of 512:
  mm1 x3: args[128, 512] = coefT @ feat            (PSUM, fp32)
  ACT:    e = Exp(args + gamma_bias) -> SBUF bf16  (1 cycle/point for ALL 128
          columns at once: ~27us/core vs 328us for the v1 row layout)
  mm2 x12: p[128, 1] = eT[:, 128c:128c+128] @ sign (signed column reduction;
          e-chunk as stationary => p-values land across 128 partitions,
          accumulated into one persistent [128, 4*NG] PSUM half-bank)
finale: one Ln over the p PSUM tile, free-dim reduce, partition-reduce via a
tiny PE matmul against ones (bf16 + bf16-error-compensation columns).
feat = 12-row bf16-split feature matrix [12, M_core] built by 2-phase scatter
DMAs from [128, 256] tiles (3 split-pair products give fp32-accurate args;
gamma enters via the ACT bias in full fp32). One preloaded ACT table set
(exp+ln+square) avoids mid-kernel table switches.
Measured (cost-model timeline, matches harness baseline methodology):
~50us/core vs 458us for the v1 baseline.
"""
import math
import sys

import numpy as np

sys.path.insert(0, "/opt/trn_rl_repo")

import ml_dtypes  # noqa: E402

LOG_GAMMA_3_2 = math.log(math.gamma(1.5))
S0 = 20.0
NCOL = 128          # column slots per pass
NCORES = 8
SPLIT_PAIRS = [(0, 0), (0, 1), (1, 0)]  # (data_i, coef_j) bf16 products
NROW = 4 * len(SPLIT_PAIRS)            # 12 feature rows; gamma via ACT bias

SCATTER_ENGINES = ["sync", "gpsimd"]   # DMA queues for pack scatters (late phases)
SCATTER_ENGINES_A = ["sync", "gpsimd", "scalar"]  # queues for first phase
LOAD_ENGINES = [("sync", None), ("scalar", None)]  # (u, v) load queue pairs
SCATTER_SPLITS = [32]                  # partition split points for scatter phases

_SEGS = [2, 3, 4, 2, 3, 2]       # per-interface x-segments (tuned, tune_dev.py)
_KS = [4, 4, 4, 4, 4, 3]         # per-interface quadrature order

_ERF = np.vectorize(math.erf, otypes=[np.float64])


# ----------------------------------------------------------------- host math
def _bf16_split3(x):
    x = np.asarray(x, np.float32)
    d1 = x.astype(ml_dtypes.bfloat16).astype(np.float32)
    r1 = (x - d1).astype(np.float32)
    d2 = r1.astype(ml_dtypes.bfloat16).astype(np.float32)
    r2 = (r1 - d2).astype(np.float32)
    d3 = r2.astype(ml_dtypes.bfloat16).astype(np.float32)
    return [d1, d2, d3]


def _consts(uniform_eps, I, W, sigma_b, sigma_n, d, r):
    n_phases = I.shape[0]
    n_int, N = uniform_eps.shape
    rho = np.tanh(np.float64(r))
    sn2 = np.float64(sigma_n) ** 2 * (1.0 - rho)
    sig_eff = np.float64(sigma_n) * np.sqrt(1.0 - rho)
    sn_sq = np.float64(sigma_n) ** 2
    logW = np.asarray(W, np.float64)
    log_w = logW - (np.log(np.sum(np.exp(logW - logW.max()))) + logW.max())
    CONST = (-np.log(np.float64(sigma_n)) - 0.5 * np.log(2 * np.pi)
             - 0.5 * np.log(sn2) - 0.5 * np.log(np.pi))
    return dict(n_phases=n_phases, n_int=n_int, N=N, sn2=sn2, sn_sq=sn_sq,
                sig_eff=sig_eff, log_w=log_w, CONST=CONST,
                sigma_b=np.float64(sigma_b), sigma_n=np.float64(sigma_n),
                d=np.float64(d))


def _gauss_quad_discrete(x, K):
    """K-node Gauss quadrature wrt sum_n delta_{x_n} (Stieltjes+Golub-Welsch)."""
    x = np.asarray(x, np.float64)
    n = len(x)
    if n <= K:
        return x, np.ones(n)
    lo, hi = x.min(), x.max()
    if hi - lo < 1e-12:
        return np.array([x.mean()]), np.array([float(n)])
    t = (2.0 * x - (lo + hi)) / (hi - lo)
    a = np.zeros(K)
    b = np.zeros(K)
    p_prev = np.zeros(n)
    p = np.ones(n)
    b[0] = n
    nrm2 = float(n)
    for k in range(K):
        a[k] = np.dot(t * p, p) / nrm2
        if k == K - 1:
            break
        q = (t - a[k]) * p - (b[k] * p_prev if k > 0 else 0.0)
        nrm2_new = np.dot(q, q)
        b[k + 1] = nrm2_new / nrm2
        p_prev, p = p, q
        nrm2 = nrm2_new
    J = np.diag(a) + np.diag(np.sqrt(b[1:K]), 1) + np.diag(np.sqrt(b[1:K]), -1)
    evals, evecs = np.linalg.eigh(J)
    w = b[0] * (evecs[0, :] ** 2)
    nodes = (evals * (hi - lo) + (lo + hi)) / 2.0
    return nodes, w


def _iface_cols_from_nodes(x, w, Ia, Ib, logw_j, cst):
    In = (_ERF(x) + 1.0) * 0.5 * (Ib - Ia) + Ia
    G = (Ib - Ia) / np.sqrt(2.0 * np.pi * cst["sigma_b"] ** 2) * np.exp(-(x ** 2))
    alpha = In / cst["sn_sq"]
    beta = 2.0 * G / cst["sn2"]
    gamma = (cst["CONST"] - np.log(G) - 0.5 * In ** 2 / cst["sn_sq"]
             - G ** 2 / cst["sn2"] + logw_j - np.log(cst["N"]) + np.log(w) + S0)
    return alpha, beta, gamma


def _build_columns(uniform_eps, I, W, sigma_b, sigma_n, d, r, segs, Ks):
    """Compressed signed column set [interior | pos | neg]. Returns dict."""
    cst = _consts(uniform_eps, I, W, sigma_b, sigma_n, d, r)
    n_phases, n_int = cst["n_phases"], cst["n_int"]
    ia, ib = np.triu_indices(n_phases, k=1)
    pos = []
    for j in range(n_int):
        eps = np.asarray(uniform_eps[j], np.float64)
        x = np.sort((2.0 * eps - 1.0) * cst["d"] / np.sqrt(2.0))
        S, K = segs[j], Ks[j]
        edges = np.linspace(x[0] - 1e-9, x[-1] + 1e-9, S + 1)
        nodes_l, wts_l = [], []
        for s in range(S):
            m = (x >= edges[s]) & (x < edges[s + 1])
            if not m.any():
                continue
            nd, wt = _gauss_quad_discrete(x[m], K)
            nodes_l.append(nd)
            wts_l.append(wt)
        nodes = np.concatenate(nodes_l)
        wts = np.concatenate(wts_l)
        Ia, Ib = np.float64(I[ia[j]]), np.float64(I[ib[j]])
        pos.append(_iface_cols_from_nodes(nodes, wts, Ia, Ib,
                                          cst["log_w"][n_phases + j], cst))
    sn_sq = cst["sn_sq"]
    beta_int_const = (np.log(2.0) - LOG_GAMMA_3_2 - 3.0 * np.log(cst["sig_eff"])
                      - np.log(cst["sigma_n"]) - 0.5 * np.log(2 * np.pi))
    I64 = np.asarray(I, np.float64)
    al_i = I64 / sn_sq
    ga_i = beta_int_const + cst["log_w"][:n_phases] - 0.5 * I64 ** 2 / sn_sq + S0
    al = np.concatenate([al_i] + [c[0] for c in pos] * 2)
    be = np.concatenate([np.zeros_like(al_i)] + [c[1] for c in pos]
                        + [-c[1] for c in pos])
    ga = np.concatenate([ga_i] + [c[2] for c in pos] * 2)
    npos = sum(len(c[0]) for c in pos)
    sign = np.concatenate([np.ones(n_phases + npos), -np.ones(npos)])
    lnv = np.concatenate([2.0 * np.ones(n_phases), np.ones(2 * npos)])
    return dict(alpha=al, beta=be, gamma=ga, sign=sign, lnv=lnv,
                sn_sq=cst["sn_sq"], sn2=cst["sn2"])


def _raw_columns(uniform_eps, I, W, sigma_b, sigma_n, d, r):
    """Uncompressed (exact) column set — segments of <=K samples stay raw."""
    n_int, N = uniform_eps.shape
    return _build_columns(uniform_eps, I, W, sigma_b, sigma_n, d, r,
                          [N] * n_int, [8] * n_int)


def _probe_validate(cols_c, cols_f, u, v, n_probe=4096):
    """(|mean d lnp|, max |d lnp|) between compressed and full sets."""
    idx = np.linspace(0, len(u) - 1, n_probe).astype(np.int64)
    uu = np.asarray(u, np.float64)[idx][:, None]
    vv = np.asarray(v, np.float64)[idx][:, None]

    def lnp(c):
        q = 0.5 * uu ** 2 / c["sn_sq"] + vv ** 2 / c["sn2"]
        arg = (uu * c["alpha"][None, :] + vv * c["beta"][None, :]
               + np.log(vv) * c["lnv"][None, :] - q + c["gamma"][None, :])
        p = np.sum(c["sign"][None, :] * np.exp(arg), axis=1)
        if (p <= 0).any():
            return None
        return np.log(p)

    a, b = lnp(cols_c), lnp(cols_f)
    if a is None or b is None:
        return np.inf, np.inf
    d = a - b
    return abs(d.mean()), np.abs(d).max()


# -------------------------------------------------------- device inputs
def _pack_coef_input(cols, n_pass):
    """[27, n_pass*NCOL] bf16 stationary + [128, n_pass] bf16 sign."""
    ncol_tot = len(cols["alpha"])
    coef_rows = np.zeros((5, n_pass * NCOL), np.float64)
    sign_full = np.ones(n_pass * NCOL, np.float64)
    coef_rows[3, :] = -1.0       # q' coefficient everywhere (incl. dead slots)
    coef_rows[4, :] = -100.0     # dead slots: exp(-100 - q') == 0
    coef_rows[0, :ncol_tot] = cols["alpha"]
    coef_rows[1, :ncol_tot] = cols["beta"]
    coef_rows[2, :ncol_tot] = cols["lnv"]
    coef_rows[4, :ncol_tot] = cols["gamma"]
    sign_full[:ncol_tot] = cols["sign"]

    npair = len(SPLIT_PAIRS)
    out = np.zeros((NROW, n_pass * NCOL), np.float32)
    for f in range(4):
        sp = _bf16_split3(coef_rows[f])
        for slot, (_, cj) in enumerate(SPLIT_PAIRS):
            out[f * npair + slot] = sp[cj]
    coef_in = out.astype(ml_dtypes.bfloat16)

    sign_in = np.zeros((128, n_pass), np.float32)
    gamma_in = np.zeros((128, n_pass), np.float32)
    for p in range(n_pass):
        sign_in[:, p] = sign_full[p * NCOL:(p + 1) * NCOL]
        gamma_in[:, p] = coef_rows[4, p * NCOL:(p + 1) * NCOL]
    return coef_in, sign_in.astype(ml_dtypes.bfloat16), gamma_in


# -------------------------------------------------------------- device kernel
def _build_core_kernel(nc, M_core, sn_sq, sn2, n_pass=1, repeat=1):
    """repeat: main-loop repetitions (timing variants). repeat=0 skips the
    main loop (prep+finale only); repeat=-1 builds an I/O-only null kernel."""
    import concourse.bass as bass  # noqa: F401
    import concourse.tile as tile
    from concourse import mybir

    F32 = mybir.dt.float32
    BF16 = mybir.dt.bfloat16
    EXP = mybir.ActivationFunctionType.Exp
    LN = mybir.ActivationFunctionType.Ln
    SQUARE = mybir.ActivationFunctionType.Square
    ADD = mybir.AluOpType.add
    SUB = mybir.AluOpType.subtract

    W = M_core // 128            # 256 cols per feature tile
    NG = M_core // 512           # groups of 512 points (64)
    assert M_core % 512 == 0

    d_u = nc.dram_tensor("u", [M_core], F32, kind="ExternalInput")
    d_v = nc.dram_tensor("v", [M_core], F32, kind="ExternalInput")
    d_coef = nc.dram_tensor("coef", [NROW, n_pass * NCOL], BF16, kind="ExternalInput")
    d_sign = nc.dram_tensor("sign", [128, n_pass], BF16, kind="ExternalInput")
    d_gamma = nc.dram_tensor("gamma", [128, n_pass], F32, kind="ExternalInput")
    d_out = nc.dram_tensor("out", [1, 1], F32, kind="ExternalOutput")

    if repeat < 0:  # I/O-only null kernel for overhead calibration
        with tile.TileContext(nc) as tc0:
            with tc0.tile_pool(name="nul", bufs=1) as nul:
                t0 = nul.tile([1, 1], F32)
                nc.vector.memset(t0, 0.0)
                nc.gpsimd.dma_start(out=d_out[:, :], in_=t0)
        return nc

    inv_sqrt2_sn = float(1.0 / math.sqrt(2.0 * sn_sq))
    inv_sqrt_sn2 = float(1.0 / math.sqrt(sn2))

    from concourse.hw_specs import get_activation_tables
    need = {EXP, LN, SQUARE}
    act_sets = list(get_activation_tables(nc.m.arch).items())
    combined_id = next((i for i, (_, s) in enumerate(act_sets) if need <= s),
                       None)

    with tile.TileContext(nc) as tc:
        if combined_id is not None:
            nc.scalar.add_instruction(mybir.InstLoadActFuncSet(
                name="preload_act_tables", act_func_set_id=combined_id,
                ins=[], outs=[]))
        with tc.tile_pool(name="const", bufs=1) as constp, \
             tc.tile_pool(name="prep", bufs=1) as prep, \
             tc.tile_pool(name="packp", bufs=1) as packp, \
             tc.tile_pool(name="esb", bufs=3) as esbp, \
             tc.tile_pool(name="fin", bufs=1) as finp, \
             tc.tile_pool(name="psA", bufs=2, space="PSUM") as psAp, \
             tc.tile_pool(name="psB", bufs=1, space="PSUM") as psBp, \
             tc.tile_pool(name="psT", bufs=1, space="PSUM") as psTp:

            coef_sb = constp.tile([NROW, n_pass * NCOL], BF16)
            nc.gpsimd.dma_start(out=coef_sb, in_=d_coef[:, :])
            sign_sb = constp.tile([128, n_pass], BF16)
            nc.gpsimd.dma_start(out=sign_sb, in_=d_sign[:, :])
            gamma_sb = constp.tile([128, n_pass], F32)
            nc.gpsimd.dma_start(out=gamma_sb, in_=d_gamma[:, :])

            # ---- features in [128, W] layout ----
            u2d = prep.tile([128, W], F32)
            v2d = prep.tile([128, W], F32)
            u_ap = d_u[:].rearrange("(p w) -> p w", w=W)
            v_ap = d_v[:].rearrange("(p w) -> p w", w=W)
            for (eng0, eng1), dst, ap in [(LOAD_ENGINES[0], u2d, u_ap),
                                          (LOAD_ENGINES[1], v2d, v_ap)]:
                e0, e1 = eng0, eng1
                if e1 is None:
                    getattr(nc, e0).dma_start(out=dst, in_=ap)
                else:
                    getattr(nc, e0).dma_start(out=dst[0:64, :], in_=ap[0:64, :])
                    getattr(nc, e1).dma_start(out=dst[64:128, :], in_=ap[64:128, :])
            lv = prep.tile([128, W], F32)
            nc.scalar.activation(out=lv, in_=v2d, func=LN)
            s1 = prep.tile([128, W], F32)
            nc.scalar.activation(out=s1, in_=u2d, func=SQUARE, scale=inv_sqrt2_sn)
            s2 = prep.tile([128, W], F32)
            nc.scalar.activation(out=s2, in_=v2d, func=SQUARE, scale=inv_sqrt_sn2)
            qp = prep.tile([128, W], F32)
            nc.vector.tensor_tensor(out=qp, in0=s1, in1=s2, op=ADD)

            # ---- pack [NROW, M_core] bf16 via scatter DMAs ----
            pack = packp.tile([NROW, M_core], BF16)
            eng_map = {"sync": nc.sync, "scalar": nc.scalar, "gpsimd": nc.gpsimd}
            phase_engines = {0: [eng_map[e] for e in SCATTER_ENGINES_A],
                             1: [eng_map[e] for e in SCATTER_ENGINES]}
            n_dma = 0

            ph_splits = [0] + SCATTER_SPLITS + [128]

            def scatter(row, src, half):
                nonlocal n_dma
                engs = phase_engines.get(half, phase_engines[1])
                p0, p1 = ph_splits[half], ph_splits[half + 1]
                engs[n_dma % len(engs)].dma_start(
                    out=pack[row:row + 1, p0 * W:p1 * W],
                    in_=src[p0:p1, :])
                n_dma += 1

            npair = len(SPLIT_PAIRS)
            split_srcs = {}
            for fi, feat in enumerate([u2d, v2d, lv, qp]):
                d1 = prep.tile([128, W], BF16, tag=f"d1_{fi}")
                nc.vector.tensor_copy(out=d1, in_=feat)
                r1 = prep.tile([128, W], F32, tag=f"r1_{fi}")
                nc.vector.tensor_tensor(out=r1, in0=feat, in1=d1, op=SUB)
                d2 = prep.tile([128, W], BF16, tag=f"d2_{fi}")
                nc.vector.tensor_copy(out=d2, in_=r1)
                splits = [d1, d2]
                for slot, (di, _) in enumerate(SPLIT_PAIRS):
                    row = fi * npair + slot
                    split_srcs[row] = splits[di]
                    scatter(row, splits[di], 0)  # phase A asap
            for half in range(1, len(ph_splits) - 1):
                for row, src in split_srcs.items():
                    scatter(row, src, half)

            # ---- main loop: NG groups of 512 points, batches of GB ----
            GB = 3
            psB = psBp.tile([128, 4 * NG], F32)   # p-values, col = 4*g + c
            if repeat == 0:
                nc.vector.memset(psB, 1.0)
            batches = []
            for rep in range(repeat):
                for b0 in range(0, NG, GB):
                    gs = list(range(b0, min(b0 + GB, NG)))
                    for p in range(n_pass):
                        batches.append((gs, p))

            psA_tiles = {}

            def emit_mm1s(k):
                gs, p = batches[k]
                psA = psAp.tile([128, 512 * GB], F32, tag="args")
                psA_tiles[k] = psA
                for i, g in enumerate(gs):
                    nc.tensor.matmul(
                        out=psA[:, 512 * i:512 * (i + 1)],
                        lhsT=coef_sb[0:NROW, p * NCOL:(p + 1) * NCOL],
                        rhs=pack[0:NROW, 512 * g:512 * (g + 1)],
                        start=True, stop=True)

            if batches:
                emit_mm1s(0)
            for k, (gs, p) in enumerate(batches):
                psA = psA_tiles.pop(k)
                bs = len(gs)
                e_sb = esbp.tile([128, 512 * GB], BF16, tag="e")
                nc.scalar.activation(out=e_sb[:, 0:512 * bs],
                                     in_=psA[:, 0:512 * bs], func=EXP,
                                     bias=gamma_sb[:, p:p + 1])
                if k + 1 < len(batches):
                    emit_mm1s(k + 1)
                for i, g in enumerate(gs):
                    for c in range(4):
                        nc.tensor.matmul(
                            out=psB[:, 4 * g + c:4 * g + c + 1],
                            lhsT=e_sb[:, 512 * i + 128 * c:512 * i + 128 * (c + 1)],
                            rhs=sign_sb[:, p:p + 1],
                            start=(p == 0), stop=(p == n_pass - 1))

            # ---- finale: Ln, free-dim reduce, partition-reduce via PE ----
            ones_col = constp.tile([128, 1], BF16)
            nc.vector.memset(ones_col, 1.0)
            lnp = finp.tile([128, 4 * NG], F32)
            nc.scalar.activation(out=lnp, in_=psB, func=LN)
            rsum = finp.tile([128, 1], BF16)
            rsum32 = finp.tile([128, 1], F32)
            nc.vector.tensor_reduce(out=rsum32, in_=lnp, op=ADD,
                                    axis=mybir.AxisListType.X)
            nc.vector.tensor_copy(out=rsum, in_=rsum32)
            err = finp.tile([128, 1], F32)
            nc.vector.tensor_tensor(out=err, in0=rsum32, in1=rsum, op=SUB)
            err_b = finp.tile([128, 1], BF16)
            nc.vector.tensor_copy(out=err_b, in_=err)
            ps_tot = psTp.tile([1, 1], F32, tag="tot")
            nc.tensor.matmul(out=ps_tot, lhsT=rsum, rhs=ones_col,
                             start=True, stop=False)
            nc.tensor.matmul(out=ps_tot, lhsT=err_b, rhs=ones_col,
                             start=False, stop=True)
            total = finp.tile([1, 1], F32)
            nc.vector.tensor_copy(out=total, in_=ps_tot)
            nc.sync.dma_start(out=d_out[:, :], in_=total)
    return nc


# ----------------------------------------------------------------- entrypoint
_kernel_cache = {}


def _choose_columns(args, u, v):
    cols = _build_columns(*args, segs=_SEGS, Ks=_KS)
    full = _raw_columns(*args)
    mean_e, max_e = _probe_validate(cols, full, u, v)
    if mean_e <= 3e-3 and max_e <= 0.3:
        return cols
    segs = list(_SEGS)
    for _ in range(3):
        segs = [s * 2 for s in segs]
        cols = _build_columns(*args, segs=segs, Ks=_KS)
        if len(cols["alpha"]) > 8 * NCOL:
            break
        mean_e, max_e = _probe_validate(cols, full, u, v)
        if mean_e <= 3e-3 and max_e <= 0.3:
            return cols
    return full


def kernel(u, v, uniform_eps, I, W, sigma_b, sigma_n, d, r):
    import jax
    import concourse.bacc as bacc
    from concourse.bass_utils import run_bass_kernel_spmd

    platforms = {dev.platform for dev in jax.devices()}
    if platforms == {"cpu"}:
        raise RuntimeError("No neuron/axon devices visible to JAX")

    u = np.asarray(u, np.float32)
    v = np.asarray(v, np.float32)
    M = u.shape[0]
    MC = M // NCORES

    args = (np.asarray(uniform_eps), np.asarray(I), np.asarray(W),
            np.asarray(sigma_b), np.asarray(sigma_n), np.asarray(d),
            np.asarray(r))
    cols = _choose_columns(args, u, v)

    ncol_tot = len(cols["alpha"])
    n_pass = (ncol_tot + NCOL - 1) // NCOL
    coef_in, sign_in, gamma_in = _pack_coef_input(cols, n_pass)

    key = (MC, n_pass)
    if key not in _kernel_cache:
        nc = bacc.Bacc()
        _build_core_kernel(nc, MC, float(cols["sn_sq"]), float(cols["sn2"]),
                           n_pass)
        nc.finalize()
        _kernel_cache[key] = nc
    nc = _kernel_cache[key]

    in_maps = [{"u": u[c * MC:(c + 1) * MC], "v": v[c * MC:(c + 1) * MC],
                "coef": coef_in, "sign": sign_in, "gamma": gamma_in}
               for c in range(NCORES)]
    res = run_bass_kernel_spmd(nc, in_maps, list(range(NCORES)))
    total = sum(float(res.results[c]["out"][0, 0]) for c in range(NCORES))
    nll = S0 - total / M
    return np.float32(nll)


# revision 3
# speedup vs baseline: 7.0646x; 1.0041x over previous
"""Trainium2 Bass kernel v2 for the BIMM2D mixture NLL (transposed layout).

Math: nll = S0 - mean_m ln p[m],  p[m] = sum_c s_c exp(arg[m,c]),
arg[m,c] = alpha_c*u + beta_c*v + l_c*ln v - q'(u,v) + gamma_c.
The 1536 interface MC columns are compressed to 124 quadrature columns
(4 interior + 62 pos + 62 neg): per interface, per x-segment, the empirical
MC sample measure is replaced by a K-node Gauss quadrature (Stieltjes +
Golub-Welsch); pos/neg blocks share nodes so the small-v pos/neg cancellation
keeps its accuracy. Validated on host (fp64 + device-precision sim) at build
time via a probe comparison against the exact column set, with automatic
segment-doubling / raw-set fallback (n_pass > 1) if the probe fails.

Device layout (transposed vs the row-major v1): coef [12, NCOL=128] is the
PE stationary; data streams as the moving operand, 3-group batches of 512:
  mm1 x3: args[128, 512] = coefT @ feat            (PSUM, fp32)
  ACT:    e = Exp(args + gamma_bias) -> SBUF bf16  (1 cycle/point for ALL 128
          columns at once: ~27us/core vs 328us for the v1 row layout)
  mm2 x12: p[128, 1] = eT[:, 128c:128c+128] @ sign (signed column reduction;
          e-chunk as stationary => p-values land across 128 partitions,
          accumulated into one persistent [128, 4*NG] PSUM half-bank)
finale: one Ln over the p PSUM tile, free-dim reduce, partition-reduce via a
tiny PE matmul against ones (bf16 + bf16-error-compensation columns).
feat = 12-row bf16-split feature matrix [12, M_core] built by 2-phase scatter
DMAs from [128, 256] tiles (3 split-pair products give fp32-accurate args;
gamma enters via the ACT bias in full fp32). One preloaded ACT table set
(exp+ln+square) avoids mid-kernel table switches.
Measured (cost-model timeline, matches harness baseline methodology):
~50us/core vs 458us for the v1 baseline.
"""
import math
import sys

import numpy as np

sys.path.insert(0, "/opt/trn_rl_repo")

import ml_dtypes  # noqa: E402

LOG_GAMMA_3_2 = math.log(math.gamma(1.5))
S0 = 20.0
NCOL = 128          # column slots per pass
NCORES = 8
SPLIT_PAIRS = [(0, 0), (0, 1), (1, 0)]  # (data_i, coef_j) bf16 products
NROW = 4 * len(SPLIT_PAIRS)            # 12 feature rows; gamma via ACT bias

SCATTER_ENGINES = ["sync", "gpsimd"]   # DMA queues for pack scatters (late phases)
SCATTER_ENGINES_A = ["sync", "gpsimd", "scalar"]  # queues for first phase
LOAD_ENGINES = [("sync", None), ("scalar", None)]  # (u, v) load queue pairs
SCATTER_SPLITS = [40]                  # partition split points for scatter phases

_SEGS = [2, 3, 4, 2, 3, 2]       # per-interface x-segments (tuned, tune_dev.py)
_KS = [4, 4, 4, 4, 4, 3]         # per-interface quadrature order

_ERF = np.vectorize(math.erf, otypes=[np.float64])


# ----------------------------------------------------------------- host math
def _bf16_split3(x):
    x = np.asarray(x, np.float32)
    d1 = x.astype(ml_dtypes.bfloat16).astype(np.float32)
    r1 = (x - d1).astype(np.float32)
    d2 = r1.astype(ml_dtypes.bfloat16).astype(np.float32)
    r2 = (r1 - d2).astype(np.float32)
    d3 = r2.astype(ml_dtypes.bfloat16).astype(np.float32)
    return [d1, d2, d3]


def _consts(uniform_eps, I, W, sigma_b, sigma_n, d, r):
    n_phases = I.shape[0]
    n_int, N = uniform_eps.shape
    rho = np.tanh(np.float64(r))
    sn2 = np.float64(sigma_n) ** 2 * (1.0 - rho)
    sig_eff = np.float64(sigma_n) * np.sqrt(1.0 - rho)
    sn_sq = np.float64(sigma_n) ** 2
    logW = np.asarray(W, np.float64)
    log_w = logW - (np.log(np.sum(np.exp(logW - logW.max()))) + logW.max())
    CONST = (-np.log(np.float64(sigma_n)) - 0.5 * np.log(2 * np.pi)
             - 0.5 * np.log(sn2) - 0.5 * np.log(np.pi))
    return dict(n_phases=n_phases, n_int=n_int, N=N, sn2=sn2, sn_sq=sn_sq,
                sig_eff=sig_eff, log_w=log_w, CONST=CONST,
                sigma_b=np.float64(sigma_b), sigma_n=np.float64(sigma_n),
                d=np.float64(d))


def _gauss_quad_discrete(x, K):
    """K-node Gauss quadrature wrt sum_n delta_{x_n} (Stieltjes+Golub-Welsch)."""
    x = np.asarray(x, np.float64)
    n = len(x)
    if n <= K:
        return x, np.ones(n)
    lo, hi = x.min(), x.max()
    if hi - lo < 1e-12:
        return np.array([x.mean()]), np.array([float(n)])
    t = (2.0 * x - (lo + hi)) / (hi - lo)
    a = np.zeros(K)
    b = np.zeros(K)
    p_prev = np.zeros(n)
    p = np.ones(n)
    b[0] = n
    nrm2 = float(n)
    for k in range(K):
        a[k] = np.dot(t * p, p) / nrm2
        if k == K - 1:
            break
        q = (t - a[k]) * p - (b[k] * p_prev if k > 0 else 0.0)
        nrm2_new = np.dot(q, q)
        b[k + 1] = nrm2_new / nrm2
        p_prev, p = p, q
        nrm2 = nrm2_new
    J = np.diag(a) + np.diag(np.sqrt(b[1:K]), 1) + np.diag(np.sqrt(b[1:K]), -1)
    evals, evecs = np.linalg.eigh(J)
    w = b[0] * (evecs[0, :] ** 2)
    nodes = (evals * (hi - lo) + (lo + hi)) / 2.0
    return nodes, w


def _iface_cols_from_nodes(x, w, Ia, Ib, logw_j, cst):
    In = (_ERF(x) + 1.0) * 0.5 * (Ib - Ia) + Ia
    G = (Ib - Ia) / np.sqrt(2.0 * np.pi * cst["sigma_b"] ** 2) * np.exp(-(x ** 2))
    alpha = In / cst["sn_sq"]
    beta = 2.0 * G / cst["sn2"]
    gamma = (cst["CONST"] - np.log(G) - 0.5 * In ** 2 / cst["sn_sq"]
             - G ** 2 / cst["sn2"] + logw_j - np.log(cst["N"]) + np.log(w) + S0)
    return alpha, beta, gamma


def _build_columns(uniform_eps, I, W, sigma_b, sigma_n, d, r, segs, Ks):
    """Compressed signed column set [interior | pos | neg]. Returns dict."""
    cst = _consts(uniform_eps, I, W, sigma_b, sigma_n, d, r)
    n_phases, n_int = cst["n_phases"], cst["n_int"]
    ia, ib = np.triu_indices(n_phases, k=1)
    pos = []
    for j in range(n_int):
        eps = np.asarray(uniform_eps[j], np.float64)
        x = np.sort((2.0 * eps - 1.0) * cst["d"] / np.sqrt(2.0))
        S, K = segs[j], Ks[j]
        edges = np.linspace(x[0] - 1e-9, x[-1] + 1e-9, S + 1)
        nodes_l, wts_l = [], []
        for s in range(S):
            m = (x >= edges[s]) & (x < edges[s + 1])
            if not m.any():
                continue
            nd, wt = _gauss_quad_discrete(x[m], K)
            nodes_l.append(nd)
            wts_l.append(wt)
        nodes = np.concatenate(nodes_l)
        wts = np.concatenate(wts_l)
        Ia, Ib = np.float64(I[ia[j]]), np.float64(I[ib[j]])
        pos.append(_iface_cols_from_nodes(nodes, wts, Ia, Ib,
                                          cst["log_w"][n_phases + j], cst))
    sn_sq = cst["sn_sq"]
    beta_int_const = (np.log(2.0) - LOG_GAMMA_3_2 - 3.0 * np.log(cst["sig_eff"])
                      - np.log(cst["sigma_n"]) - 0.5 * np.log(2 * np.pi))
    I64 = np.asarray(I, np.float64)
    al_i = I64 / sn_sq
    ga_i = beta_int_const + cst["log_w"][:n_phases] - 0.5 * I64 ** 2 / sn_sq + S0
    al = np.concatenate([al_i] + [c[0] for c in pos] * 2)
    be = np.concatenate([np.zeros_like(al_i)] + [c[1] for c in pos]
                        + [-c[1] for c in pos])
    ga = np.concatenate([ga_i] + [c[2] for c in pos] * 2)
    npos = sum(len(c[0]) for c in pos)
    sign = np.concatenate([np.ones(n_phases + npos), -np.ones(npos)])
    lnv = np.concatenate([2.0 * np.ones(n_phases), np.ones(2 * npos)])
    return dict(alpha=al, beta=be, gamma=ga, sign=sign, lnv=lnv,
                sn_sq=cst["sn_sq"], sn2=cst["sn2"])


def _raw_columns(uniform_eps, I, W, sigma_b, sigma_n, d, r):
    """Uncompressed (exact) column set — segments of <=K samples stay raw."""
    n_int, N = uniform_eps.shape
    return _build_columns(uniform_eps, I, W, sigma_b, sigma_n, d, r,
                          [N] * n_int, [8] * n_int)


def _probe_validate(cols_c, cols_f, u, v, n_probe=4096):
    """(|mean d lnp|, max |d lnp|) between compressed and full sets."""
    idx = np.linspace(0, len(u) - 1, n_probe).astype(np.int64)
    uu = np.asarray(u, np.float64)[idx][:, None]
    vv = np.asarray(v, np.float64)[idx][:, None]

    def lnp(c):
        q = 0.5 * uu ** 2 / c["sn_sq"] + vv ** 2 / c["sn2"]
        arg = (uu * c["alpha"][None, :] + vv * c["beta"][None, :]
               + np.log(vv) * c["lnv"][None, :] - q + c["gamma"][None, :])
        p = np.sum(c["sign"][None, :] * np.exp(arg), axis=1)
        if (p <= 0).any():
            return None
        return np.log(p)

    a, b = lnp(cols_c), lnp(cols_f)
    if a is None or b is None:
        return np.inf, np.inf
    d = a - b
    return abs(d.mean()), np.abs(d).max()


# -------------------------------------------------------- device inputs
def _pack_coef_input(cols, n_pass):
    """[27, n_pass*NCOL] bf16 stationary + [128, n_pass] bf16 sign."""
    ncol_tot = len(cols["alpha"])
    coef_rows = np.zeros((5, n_pass * NCOL), np.float64)
    sign_full = np.ones(n_pass * NCOL, np.float64)
    coef_rows[3, :] = -1.0       # q' coefficient everywhere (incl. dead slots)
    coef_rows[4, :] = -100.0     # dead slots: exp(-100 - q') == 0
    coef_rows[0, :ncol_tot] = cols["alpha"]
    coef_rows[1, :ncol_tot] = cols["beta"]
    coef_rows[2, :ncol_tot] = cols["lnv"]
    coef_rows[4, :ncol_tot] = cols["gamma"]
    sign_full[:ncol_tot] = cols["sign"]

    npair = len(SPLIT_PAIRS)
    out = np.zeros((NROW, n_pass * NCOL), np.float32)
    for f in range(4):
        sp = _bf16_split3(coef_rows[f])
        for slot, (_, cj) in enumerate(SPLIT_PAIRS):
            out[f * npair + slot] = sp[cj]
    coef_in = out.astype(ml_dtypes.bfloat16)

    sign_in = np.zeros((128, n_pass), np.float32)
    gamma_in = np.zeros((128, n_pass), np.float32)
    for p in range(n_pass):
        sign_in[:, p] = sign_full[p * NCOL:(p + 1) * NCOL]
        gamma_in[:, p] = coef_rows[4, p * NCOL:(p + 1) * NCOL]
    return coef_in, sign_in.astype(ml_dtypes.bfloat16), gamma_in


# -------------------------------------------------------------- device kernel
def _build_core_kernel(nc, M_core, sn_sq, sn2, n_pass=1, repeat=1):
    """repeat: main-loop repetitions (timing variants). repeat=0 skips the
    main loop (prep+finale only); repeat=-1 builds an I/O-only null kernel."""
    import concourse.bass as bass  # noqa: F401
    import concourse.tile as tile
    from concourse import mybir

    F32 = mybir.dt.float32
    BF16 = mybir.dt.bfloat16
    EXP = mybir.ActivationFunctionType.Exp
    LN = mybir.ActivationFunctionType.Ln
    SQUARE = mybir.ActivationFunctionType.Square
    ADD = mybir.AluOpType.add
    SUB = mybir.AluOpType.subtract

    W = M_core // 128            # 256 cols per feature tile
    NG = M_core // 512           # groups of 512 points (64)
    assert M_core % 512 == 0

    d_u = nc.dram_tensor("u", [M_core], F32, kind="ExternalInput")
    d_v = nc.dram_tensor("v", [M_core], F32, kind="ExternalInput")
    d_coef = nc.dram_tensor("coef", [NROW, n_pass * NCOL], BF16, kind="ExternalInput")
    d_sign = nc.dram_tensor("sign", [128, n_pass], BF16, kind="ExternalInput")
    d_gamma = nc.dram_tensor("gamma", [128, n_pass], F32, kind="ExternalInput")
    d_out = nc.dram_tensor("out", [1, 1], F32, kind="ExternalOutput")

    if repeat < 0:  # I/O-only null kernel for overhead calibration
        with tile.TileContext(nc) as tc0:
            with tc0.tile_pool(name="nul", bufs=1) as nul:
                t0 = nul.tile([1, 1], F32)
                nc.vector.memset(t0, 0.0)
                nc.gpsimd.dma_start(out=d_out[:, :], in_=t0)
        return nc

    inv_sqrt2_sn = float(1.0 / math.sqrt(2.0 * sn_sq))
    inv_sqrt_sn2 = float(1.0 / math.sqrt(sn2))

    from concourse.hw_specs import get_activation_tables
    need = {EXP, LN, SQUARE}
    act_sets = list(get_activation_tables(nc.m.arch).items())
    combined_id = next((i for i, (_, s) in enumerate(act_sets) if need <= s),
                       None)

    with tile.TileContext(nc) as tc:
        if combined_id is not None:
            nc.scalar.add_instruction(mybir.InstLoadActFuncSet(
                name="preload_act_tables", act_func_set_id=combined_id,
                ins=[], outs=[]))
        with tc.tile_pool(name="const", bufs=1) as constp, \
             tc.tile_pool(name="prep", bufs=1) as prep, \
             tc.tile_pool(name="packp", bufs=1) as packp, \
             tc.tile_pool(name="esb", bufs=3) as esbp, \
             tc.tile_pool(name="fin", bufs=1) as finp, \
             tc.tile_pool(name="psA", bufs=2, space="PSUM") as psAp, \
             tc.tile_pool(name="psB", bufs=1, space="PSUM") as psBp, \
             tc.tile_pool(name="psT", bufs=1, space="PSUM") as psTp:

            coef_sb = constp.tile([NROW, n_pass * NCOL], BF16)
            nc.gpsimd.dma_start(out=coef_sb, in_=d_coef[:, :])
            sign_sb = constp.tile([128, n_pass], BF16)
            nc.gpsimd.dma_start(out=sign_sb, in_=d_sign[:, :])
            gamma_sb = constp.tile([128, n_pass], F32)
            nc.gpsimd.dma_start(out=gamma_sb, in_=d_gamma[:, :])

            # ---- features in [128, W] layout ----
            u2d = prep.tile([128, W], F32)
            v2d = prep.tile([128, W], F32)
            u_ap = d_u[:].rearrange("(p w) -> p w", w=W)
            v_ap = d_v[:].rearrange("(p w) -> p w", w=W)
            for (eng0, eng1), dst, ap in [(LOAD_ENGINES[0], u2d, u_ap),
                                          (LOAD_ENGINES[1], v2d, v_ap)]:
                e0, e1 = eng0, eng1
                if e1 is None:
                    getattr(nc, e0).dma_start(out=dst, in_=ap)
                else:
                    getattr(nc, e0).dma_start(out=dst[0:64, :], in_=ap[0:64, :])
                    getattr(nc, e1).dma_start(out=dst[64:128, :], in_=ap[64:128, :])
            lv = prep.tile([128, W], F32)
            nc.scalar.activation(out=lv, in_=v2d, func=LN)
            s1 = prep.tile([128, W], F32)
            nc.scalar.activation(out=s1, in_=u2d, func=SQUARE, scale=inv_sqrt2_sn)
            s2 = prep.tile([128, W], F32)
            nc.scalar.activation(out=s2, in_=v2d, func=SQUARE, scale=inv_sqrt_sn2)
            qp = prep.tile([128, W], F32)
            nc.vector.tensor_tensor(out=qp, in0=s1, in1=s2, op=ADD)

            # ---- pack [NROW, M_core] bf16 via scatter DMAs ----
            pack = packp.tile([NROW, M_core], BF16)
            eng_map = {"sync": nc.sync, "scalar": nc.scalar, "gpsimd": nc.gpsimd}
            phase_engines = {0: [eng_map[e] for e in SCATTER_ENGINES_A],
                             1: [eng_map[e] for e in SCATTER_ENGINES]}
            n_dma = 0

            ph_splits = [0] + SCATTER_SPLITS + [128]

            def scatter(row, src, half):
                nonlocal n_dma
                engs = phase_engines.get(half, phase_engines[1])
                p0, p1 = ph_splits[half], ph_splits[half + 1]
                engs[n_dma % len(engs)].dma_start(
                    out=pack[row:row + 1, p0 * W:p1 * W],
                    in_=src[p0:p1, :])
                n_dma += 1

            npair = len(SPLIT_PAIRS)
            split_srcs = {}
            for fi, feat in enumerate([u2d, v2d, lv, qp]):
                d1 = prep.tile([128, W], BF16, tag=f"d1_{fi}")
                nc.vector.tensor_copy(out=d1, in_=feat)
                r1 = prep.tile([128, W], F32, tag=f"r1_{fi}")
                nc.vector.tensor_tensor(out=r1, in0=feat, in1=d1, op=SUB)
                d2 = prep.tile([128, W], BF16, tag=f"d2_{fi}")
                nc.vector.tensor_copy(out=d2, in_=r1)
                splits = [d1, d2]
                for slot, (di, _) in enumerate(SPLIT_PAIRS):
                    row = fi * npair + slot
                    split_srcs[row] = splits[di]
                    scatter(row, splits[di], 0)  # phase A asap
            for half in range(1, len(ph_splits) - 1):
                for row, src in split_srcs.items():
                    scatter(row, src, half)

            # ---- main loop: NG groups of 512 points, batches of GB ----
            GB = 3
            psB = psBp.tile([128, 4 * NG], F32)   # p-values, col = 4*g + c
            if repeat == 0:
                nc.vector.memset(psB, 1.0)
            batches = []
            for rep in range(repeat):
                for b0 in range(0, NG, GB):
                    gs = list(range(b0, min(b0 + GB, NG)))
                    for p in range(n_pass):
                        batches.append((gs, p))

            psA_tiles = {}

            def emit_mm1s(k):
                gs, p = batches[k]
                psA = psAp.tile([128, 512 * GB], F32, tag="args")
                psA_tiles[k] = psA
                for i, g in enumerate(gs):
                    nc.tensor.matmul(
                        out=psA[:, 512 * i:512 * (i + 1)],
                        lhsT=coef_sb[0:NROW, p * NCOL:(p + 1) * NCOL],
                        rhs=pack[0:NROW, 512 * g:512 * (g + 1)],
                        start=True, stop=True)

            if batches:
                emit_mm1s(0)
            for k, (gs, p) in enumerate(batches):
                psA = psA_tiles.pop(k)
                bs = len(gs)
                e_sb = esbp.tile([128, 512 * GB], BF16, tag="e")
                nc.scalar.activation(out=e_sb[:, 0:512 * bs],
                                     in_=psA[:, 0:512 * bs], func=EXP,
                                     bias=gamma_sb[:, p:p + 1])
                if k + 1 < len(batches):
                    emit_mm1s(k + 1)
                for i, g in enumerate(gs):
                    for c in range(4):
                        nc.tensor.matmul(
                            out=psB[:, 4 * g + c:4 * g + c + 1],
                            lhsT=e_sb[:, 512 * i + 128 * c:512 * i + 128 * (c + 1)],
                            rhs=sign_sb[:, p:p + 1],
                            start=(p == 0), stop=(p == n_pass - 1))

            # ---- finale: Ln, free-dim reduce, partition-reduce via PE ----
            ones_col = constp.tile([128, 1], BF16)
            nc.vector.memset(ones_col, 1.0)
            lnp = finp.tile([128, 4 * NG], F32)
            nc.scalar.activation(out=lnp, in_=psB, func=LN)
            rsum = finp.tile([128, 1], BF16)
            rsum32 = finp.tile([128, 1], F32)
            nc.vector.tensor_reduce(out=rsum32, in_=lnp, op=ADD,
                                    axis=mybir.AxisListType.X)
            nc.vector.tensor_copy(out=rsum, in_=rsum32)
            err = finp.tile([128, 1], F32)
            nc.vector.tensor_tensor(out=err, in0=rsum32, in1=rsum, op=SUB)
            err_b = finp.tile([128, 1], BF16)
            nc.vector.tensor_copy(out=err_b, in_=err)
            ps_tot = psTp.tile([1, 1], F32, tag="tot")
            nc.tensor.matmul(out=ps_tot, lhsT=rsum, rhs=ones_col,
                             start=True, stop=False)
            nc.tensor.matmul(out=ps_tot, lhsT=err_b, rhs=ones_col,
                             start=False, stop=True)
            total = finp.tile([1, 1], F32)
            nc.vector.tensor_copy(out=total, in_=ps_tot)
            nc.sync.dma_start(out=d_out[:, :], in_=total)
    return nc


# ----------------------------------------------------------------- entrypoint
_kernel_cache = {}


def _choose_columns(args, u, v):
    cols = _build_columns(*args, segs=_SEGS, Ks=_KS)
    full = _raw_columns(*args)
    mean_e, max_e = _probe_validate(cols, full, u, v)
    if mean_e <= 3e-3 and max_e <= 0.3:
        return cols
    segs = list(_SEGS)
    for _ in range(3):
        segs = [s * 2 for s in segs]
        cols = _build_columns(*args, segs=segs, Ks=_KS)
        if len(cols["alpha"]) > 8 * NCOL:
            break
        mean_e, max_e = _probe_validate(cols, full, u, v)
        if mean_e <= 3e-3 and max_e <= 0.3:
            return cols
    return full


def kernel(u, v, uniform_eps, I, W, sigma_b, sigma_n, d, r):
    import jax
    import concourse.bacc as bacc
    from concourse.bass_utils import run_bass_kernel_spmd

    platforms = {dev.platform for dev in jax.devices()}
    if platforms == {"cpu"}:
        raise RuntimeError("No neuron/axon devices visible to JAX")

    u = np.asarray(u, np.float32)
    v = np.asarray(v, np.float32)
    M = u.shape[0]
    MC = M // NCORES

    args = (np.asarray(uniform_eps), np.asarray(I), np.asarray(W),
            np.asarray(sigma_b), np.asarray(sigma_n), np.asarray(d),
            np.asarray(r))
    cols = _choose_columns(args, u, v)

    ncol_tot = len(cols["alpha"])
    n_pass = (ncol_tot + NCOL - 1) // NCOL
    coef_in, sign_in, gamma_in = _pack_coef_input(cols, n_pass)

    key = (MC, n_pass)
    if key not in _kernel_cache:
        nc = bacc.Bacc()
        _build_core_kernel(nc, MC, float(cols["sn_sq"]), float(cols["sn2"]),
                           n_pass)
        nc.finalize()
        _kernel_cache[key] = nc
    nc = _kernel_cache[key]

    in_maps = [{"u": u[c * MC:(c + 1) * MC], "v": v[c * MC:(c + 1) * MC],
                "coef": coef_in, "sign": sign_in, "gamma": gamma_in}
               for c in range(NCORES)]
    res = run_bass_kernel_spmd(nc, in_maps, list(range(NCORES)))
    total = sum(float(res.results[c]["out"][0, 0]) for c in range(NCORES))
    nll = S0 - total / M
    return np.float32(nll)
